# revision 95
# baseline (speedup 1.0000x reference)
"""Trainium2 Bass kernel for LoopRelationalGraphConvolution.

Math (matches the jax reference):
    out[n] = relu( SCALE * sum_s  W[rel[n,s]] @ emb[neighbors[n,s]] )
    SCALE  = 1000 / (R1 * S)      (folds the mean over S and the /R1 * 1000)

Design (8 NeuronCores, data-parallel over the 8192-node batch):
  A two-level host balancer assigns nodes to cores and then to 8 tiles of
  exactly 128 nodes per core such that every (tile, relation) bucket has
  <=128 edges (zero overflow).  The host then lays out, per tile, DENSE
  fp8 edge-embedding images in the exact SBUF layout stage-1 consumes
  (partition = dim%128, free = [slot, k-tile]), split into an e8 image
  (fp8(emb)) and an er8 image (fp8(emb - e8)).  Dense images stream at the
  full modeled DMA rate (no per-row gather floor, no GPSIMD prep chain).

  Per tile the device kernel:
    1. stage-1: per relation r, fp8 DoubleRow matmuls accumulate
       Y[slot, o] into PSUM (f32):
         all tiles:            e8 * w8
         tiles in ER8_TILES:   + er8 * w8    (corrects emb fp8 error)
         tiles not in SKIP_WR8:+ e8 * wr8    (corrects W fp8 error)
       w8/wr8 are the exact fp8 split of W*SCALE*WPRE; each DoubleRow
       contracts K=256 in 1 instr at 0.5 cycles/row.  The er8 term runs
       only where ER8_CUT covers (tiles 1-3 here): skipping it on the
       other tiles trades ~1.5% relative error (gate 2e-2) for DMA and
       PE time.  Two consecutive relations share one PSUM bank.
    2. stage-2 matmul: fp8 0/1 selection matrix reduces edge slots into
       node rows: out_psum[node, o] += SEL_r^T @ Y_bf16 (accumulated over
       all r; the evacuation folds the 1/WPRE prescale compensation).
    3. relu on PSUM->SBUF evacuation (bf16), DMA node rows to DRAM.
  Stage-1/stage-2 run as one continuous software pipeline across tiles
  (PSKEW pair-groups of skew).  The device program is fully static and
  identical across cores (SPMD); all data-dependence lives in the host-
  packed images.  Host post-step inverse-permutes rows back to the
  original node order.
"""

import numpy as np
import ml_dtypes

bf16 = ml_dtypes.bfloat16
fp8 = ml_dtypes.float8_e4m3

# Problem constants (hardcoded per contract).
V = 100000
D = 256
R1 = 33          # relations incl. self-loop
N = 8192
S = 32
NCORES = 8
NPC = N // NCORES          # 1024 nodes per core
NTILES = 8                 # node-tiles per core (perfect 128-node packing)
CAPS = [128] * NTILES      # nodes per tile (uniform across cores)
ROW_BASE = np.concatenate([[0], np.cumsum(CAPS)]).tolist()
P = 128
NSLOT = R1 * P             # 4224 edge slots per tile
SCALE = 1000.0 / (R1 * S)
WPRE = 256.0               # fp8 W prescale; undone at evac
UMAX = 32768               # compacted per-core embedding rows bound

# DMA segmentation (relation ranges) for the pipeline-head tiles and W.
GSPLIT = [0, 2, 7, 13, 19, 26, R1]
WBATCH = [0, 2, 7, 13, 19, 26, R1]

# Precision-term schedule: the er8*w8 correction term (emb fp8 residual)
# runs per (tile, relation < cut).  The head tiles run all three terms
# (the PE is in its slow p-state there anyway, so it tracks the DMA-bound
# head stream); the error budget is spent skipping er8 on the LATER
# tiles, which trims total DMA and shortens the tail.  Dropping er8 on a
# fraction f of edges costs sqrt(f)*2.25% relative error (gate: 2e-2;
# full-er8 baseline measures 0.22%).  Cuts must lie on GSPLIT boundaries.
ER8_CUT = (0, R1, R1, 19, 0, 0, 0, 0)
# The e8*wr8 term (W fp8 residual) runs per (tile, relation < cut), like
# ER8_CUT but purely a matmul-count knob (W stays loaded for other tiles,
# so no DMA change).  Skipping a relation costs sqrt(1/264)*2.2% error.
WR8_CUT = (R1, R1, R1, R1, R1, R1, R1, 21)
# Tiles whose stage-2 slot->node reduction runs as fp8 DoubleRow pair
# matmuls (Y evacuated to fp8 instead of bf16): half the stage-2 PE time
# for ~0.94% relative error on that tile's rows.  (Currently disabled:
# er8-skips buy more time per unit of error while DMA is near PE.)
S2F8_TILES = ()

# Software-pipeline skew between stage-1 and stage-2 of consecutive chunks.
PSKEW = 5   # pipeline skew in relation-PAIRS (2 relations share a PSUM bank)
NPAIR = (R1 + 1) // 2      # 17 pair-groups per tile (last is a single)
PF = 2      # tile prefetch depth


def _q8(x):
    return np.asarray(x, np.float32).astype(fp8)


# ---------------------------------------------------------------------------
# Host-side preparation
# ---------------------------------------------------------------------------

def _balance_cores(hist, rng):
    """Assign the N nodes to NCORES cores (NPC each), minimizing the max
    per-(core, relation) edge total so every core can then be split into
    NTILES tiles with <=128-edge buckets. Greedy seed + targeted swaps."""
    order = np.argsort(-hist.max(axis=1), kind="stable")
    load = np.zeros((NCORES, R1), dtype=np.int64)
    cnt = np.zeros(NCORES, dtype=np.int64)
    coreof = np.empty(N, dtype=np.int64)
    for n in order:
        h = hist[n]
        new = load + h
        key = new.max(axis=1) * 100000 + new.sum(axis=1) // 256
        key[cnt >= NPC] = 1 << 60
        best = int(np.argmin(key))
        coreof[n] = best
        load[best] += h
        cnt[best] += 1
    target = NTILES * P - 16
    stall = 0
    for _ in range(20000):
        worst = int(load.max())
        if worst <= target or stall > 2000:
            break
        cells = np.argwhere(load >= worst)
        c, r = (int(x) for x in cells[rng.integers(len(cells))])
        cand = np.nonzero((coreof == c) & (hist[:, r] > 0))[0]
        i = int(rng.choice(cand, size=1)[0])
        hi = hist[i]
        improved = False
        for c2 in np.argsort(load[:, r])[:3]:
            c2 = int(c2)
            if c2 == c:
                continue
            cand2 = np.nonzero((coreof == c2) & (hist[:, r] == 0))[0]
            if len(cand2) == 0:
                continue
            js = rng.choice(cand2, size=min(32, len(cand2)), replace=False)
            hj = hist[js]
            n1 = (load[c] - hi)[None, :] + hj
            n2 = (load[c2] + hi)[None, :] - hj
            mx = np.maximum(n1.max(axis=1), n2.max(axis=1))
            k = int(np.argmin(mx))
            if mx[k] < worst or (mx[k] == worst
                                 and n1[k].max() < load[c].max()):
                j = int(js[k])
                load[c], load[c2] = n1[k], n2[k]
                coreof[i], coreof[j] = c2, c
                improved = True
                break
        stall = 0 if improved else stall + 1
    return coreof


def _balance_tiles(hist_c, seed=0):
    """Assign NPC nodes to NTILES tiles of exactly 128 nodes with every
    (tile, relation) bucket <= 128. Greedy seed + swap search with sideways
    moves. Returns (tiles, loads); raises if no zero-overflow split found."""
    for attempt in range(16):
        rng = np.random.default_rng(seed + attempt)
        n = len(hist_c)
        order = np.argsort(-hist_c.max(axis=1), kind="stable")
        load = np.zeros((NTILES, R1), dtype=np.int64)
        cnt = np.zeros(NTILES, dtype=np.int64)
        tileof = np.empty(n, dtype=np.int64)
        soft = P - 2
        for i in order:
            h = hist_c[i]
            new = load + h
            over = np.maximum(new - soft, 0).sum(axis=1)
            key = (over * (1 << 20) + new.max(axis=1) * 2048
                   + new.sum(axis=1) // 64)
            key[cnt >= P] = 1 << 60
            t = int(np.argmin(key))
            tileof[i] = t
            load[t] += h
            cnt[t] += 1
        cur = int(np.maximum(load - P, 0).sum())
        sideways = 0
        for _ in range(8000):
            if cur == 0:
                break
            t, r = np.unravel_index(int(np.argmax(load - P)), load.shape)
            cand = np.nonzero((tileof == t) & (hist_c[:, r] > 0))[0]
            i = int(rng.choice(cand, size=1)[0])
            hi = hist_c[i]
            best = None
            for t2 in range(NTILES):
                if t2 == t:
                    continue
                cand2 = np.nonzero((tileof == t2)
                                   & (hist_c[:, r] < hi[r]))[0]
                if len(cand2) == 0:
                    continue
                js = rng.choice(cand2, size=min(48, len(cand2)),
                                replace=False)
                hj = hist_c[js]
                n1 = (load[t] - hi)[None, :] + hj
                n2 = (load[t2] + hi)[None, :] - hj
                novr = (np.maximum(n1 - P, 0).sum(axis=1)
                        + np.maximum(n2 - P, 0).sum(axis=1)
                        + np.maximum(load - P, 0).sum()
                        - np.maximum(load[t] - P, 0).sum()
                        - np.maximum(load[t2] - P, 0).sum())
                k = int(np.argmin(novr))
                if best is None or novr[k] < best[0]:
                    best = (int(novr[k]), t2, int(js[k]), n1[k].copy(),
                            n2[k].copy())
            if best is None:
                continue
            novr_k, t2, j, n1k, n2k = best
            if novr_k < cur or (novr_k == cur and sideways < 2000
                                and rng.random() < 0.5):
                if novr_k == cur:
                    sideways += 1
                load[t], load[t2] = n1k, n2k
                tileof[i], tileof[j] = t2, t
                cur = int(np.maximum(load - P, 0).sum())
        if cur == 0:
            tiles = [np.nonzero(tileof == t)[0].tolist()
                     for t in range(NTILES)]
            return tiles, load
    raise AssertionError("tile balance failed: could not reach 0 overflow")


def _img_from_rows(rows):
    """[n slots, 256] fp8 rows -> dense stage-1 image [128, 2, n] where
    img[p, c, i] = rows[i][c*128 + p] (c-major: each DoubleRow k-tile is a
    contiguous stride-1 row, as the PE Ldweights ISA requires)."""
    n = rows.shape[0]
    return np.ascontiguousarray(
        rows.reshape(n, 2, 128).transpose(2, 1, 0))


def _seg_layout(t):
    """Per-segment packing of tile t's image: list of
    (byte_base, slot_width, has_er8), plus the total used byte width."""
    cut = ER8_CUT[t]
    assert cut in GSPLIT, (t, cut)
    segs, base = [], 0
    for g in range(len(GSPLIT) - 1):
        a, b = P * GSPLIT[g], P * GSPLIT[g + 1]
        w = b - a
        has = GSPLIT[g] < cut
        segs.append((base, w, has))
        base += (4 if has else 2) * w
    return segs, base


def _pack_tile_img(t, rows8, rowsr):
    """Pack one tile's dense image into [128, 4*NSLOT] fp8: per DMA
    segment g (slots a..b), the e8 block then (if covered) the er8 block,
    each [c, i]-major, so every segment is one contiguous chunk."""
    img = np.zeros((128, 4 * NSLOT), dtype=fp8)
    segs, _ = _seg_layout(t)
    for g, (base, w, has) in enumerate(segs):
        a = P * GSPLIT[g]
        img[:, base:base + 2 * w] = (
            _img_from_rows(rows8[a:a + w]).reshape(128, 2 * w))
        if has:
            img[:, base + 2 * w:base + 4 * w] = (
                _img_from_rows(rowsr[a:a + w]).reshape(128, 2 * w))
    return img


def _img_blk(t, r, which):
    """(segment byte base, segment slot width, slot offset) of relation r's
    e8 (which=0) / er8 (which=1) block inside the packed image."""
    g = next(i for i in range(len(GSPLIT) - 1)
             if GSPLIT[i] <= r < GSPLIT[i + 1])
    segs, _ = _seg_layout(t)
    base, w, has = segs[g]
    assert which == 0 or has
    return base + which * 2 * w, w, r * P - P * GSPLIT[g]


def prep(emb_table, weights, neighbors, relations):
    """Build per-core device arrays. Returns (in_maps, perms)."""
    emb_f = np.asarray(emb_table, dtype=np.float32)
    # W' = W*SCALE*WPRE, exact-split into fp8 w8 + wr8.
    wq = np.asarray(weights, dtype=np.float32) * (SCALE * WPRE)  # [R1, O, D]
    w8 = _q8(wq)
    wr8 = _q8(wq - w8.astype(np.float32))
    # W img[p, r, c, o] = w[r, o, c*128+p]
    def wlayout(w):
        w_rdo = np.ascontiguousarray(w.transpose(0, 2, 1))      # [r, d, o]
        return np.ascontiguousarray(
            w_rdo.reshape(R1, 2, 128, D).transpose(2, 0, 1, 3))  # [p,r,c,o]
    W8 = wlayout(w8)
    WR8 = wlayout(wr8)

    neighbors = np.asarray(neighbors).astype(np.int64)
    relations = np.asarray(relations).astype(np.int64)

    ghist = np.zeros((N, R1), dtype=np.int64)
    np.add.at(ghist, (np.repeat(np.arange(N), S), relations.ravel()), 1)
    coreof = _balance_cores(ghist, np.random.default_rng(0))

    in_maps, perms = [], []
    for c in range(NCORES):
        cnodes = np.nonzero(coreof == c)[0]                   # global ids
        nb = neighbors[cnodes]                                # [NPC, S]
        rel = relations[cnodes]
        uniq, inv = np.unique(nb.ravel(), return_inverse=True)
        inv = inv.reshape(nb.shape).astype(np.int64)
        U = len(uniq)
        assert U <= UMAX, U
        ef = emb_f[uniq]
        e8 = _q8(ef)                                          # [U, 256] fp8
        er8 = _q8(ef - e8.astype(np.float32))

        tiles, loads = _balance_tiles(ghist[cnodes], seed=16 * c)
        assert loads.max() <= P, f"balance failed: max bucket {loads.max()}"

        img_all = np.zeros((NTILES, 128, 4 * NSLOT), dtype=fp8)
        # one zero relation-column of padding so fp8 stage-2 relation PAIRS
        # (DoubleRow, K=256) stay uniform for the odd final relation
        sel_all = np.zeros((NTILES, 128, (R1 + 1) * 128), dtype=fp8)
        perm = []
        for t, nodes in enumerate(tiles):
            nodes = np.array(nodes, dtype=np.int64)
            ncnt = len(nodes)
            assert ncnt == CAPS[t]
            perm.extend(cnodes[nodes].tolist())
            # edges of this tile
            er = rel[nodes].ravel()                            # relation per edge
            ei = inv[nodes].ravel()                            # compact nbr id
            ej = np.repeat(np.arange(ncnt), S)                 # local node idx
            order = np.argsort(er, kind="stable")
            er_s, ei_s, ej_s = er[order], ei[order], ej[order]
            # position within relation group
            start = np.searchsorted(er_s, np.arange(R1))
            pos = np.arange(ncnt * S) - start[er_s]
            slot = er_s * P + pos                              # [ncnt*S]
            rows8 = np.zeros((NSLOT, D), dtype=fp8)
            rows8[slot] = e8[ei_s]
            rowsr = None
            if ER8_CUT[t] > 0:
                rowsr = np.zeros((NSLOT, D), dtype=fp8)
                rowsr[slot] = er8[ei_s]
            img_all[t] = _pack_tile_img(t, rows8, rowsr)
            sel = np.zeros((NSLOT, 128), dtype=fp8)
            sel[slot, ej_s] = fp8(1.0)
            # device SEL layout: [part p = slot-in-chunk, free = r*128 + node]
            sel_all[t, :, :NSLOT] = np.ascontiguousarray(
                sel.reshape(R1, P, 128).transpose(1, 0, 2).reshape(P, NSLOT))
        in_maps.append({
            "w8i": W8,
            "wr8i": WR8,
            "img": img_all,
            "sel": np.ascontiguousarray(
                sel_all.reshape(NTILES * 128, (R1 + 1) * 128)),
        })
        perms.append(np.array(perm, dtype=np.int64))

    return in_maps, perms


# ---------------------------------------------------------------------------
# Numpy emulation (dtype-faithful) for validation
# ---------------------------------------------------------------------------

def emulate_core(in_map):
    w8 = in_map["w8i"].astype(np.float32)                      # [p, r, c, o]
    wr8 = in_map["wr8i"].astype(np.float32)
    # reconstruct [r, d, o]
    def wback(wimg):
        return wimg.transpose(1, 2, 0, 3).reshape(R1, 2 * 128, D)
    w8t, wr8t = wback(w8), wback(wr8)
    sel = in_map["sel"].reshape(NTILES, 128, (R1 + 1) * 128)
    out = np.zeros((NPC, D), dtype=np.float32)
    for t in range(NTILES):
        img = in_map["img"][t].astype(np.float32)              # [p, 4*NSLOT]

        def block(r, which):
            base, w, off = _img_blk(t, r, which)
            blk = img[:, base:base + 2 * w].reshape(128, 2, w)[:, :, off:off + P]
            # [p, c, i] -> [i, d = c*128+p]
            return blk.transpose(2, 1, 0).reshape(P, 256)

        out_acc = np.zeros((128, D), dtype=np.float32)
        for r in range(R1):
            E8r = block(r, 0)
            Y = E8r @ w8t[r]
            if r < ER8_CUT[t]:
                Y = Y + block(r, 1) @ w8t[r]
            if r < WR8_CUT[t]:
                Y = Y + E8r @ wr8t[r]
            ydt = fp8 if t in S2F8_TILES else bf16
            Yb = (Y / WPRE).astype(ydt).astype(np.float32)
            selr = sel[t][:, r * 128:(r + 1) * 128].astype(np.float32)
            out_acc += selr.T @ Yb
        base, ncnt = ROW_BASE[t], CAPS[t]
        outb = np.maximum(out_acc[:ncnt], 0.0).astype(bf16).astype(np.float32)
        out[base:base + ncnt] = outb
    return out


def emulate(emb_table, weights, neighbors, relations):
    in_maps, perms = prep(emb_table, weights, neighbors, relations)
    full = np.zeros((N, D), dtype=np.float32)
    for c in range(NCORES):
        full[perms[c]] = emulate_core(in_maps[c])
    return full


# ---------------------------------------------------------------------------
# Bass program
# ---------------------------------------------------------------------------

def build_program():
    import concourse.bacc as bacc
    import concourse.tile as tile
    import concourse.mybir as mybir

    nc = bacc.Bacc(
        "TRN2", target_bir_lowering=False, debug=False,
        num_devices=NCORES,
    )
    BF = mybir.dt.bfloat16
    F32 = mybir.dt.float32
    F8 = mybir.dt.float8e4
    DR = mybir.MatmulPerfMode.DoubleRow

    w8i = nc.dram_tensor("w8i", [128, R1, 2, D], F8, kind="ExternalInput").ap()
    wr8i = nc.dram_tensor("wr8i", [128, R1, 2, D], F8,
                          kind="ExternalInput").ap()
    img = nc.dram_tensor("img", [NTILES, 128, 4 * NSLOT], F8,
                         kind="ExternalInput").ap()
    sel = nc.dram_tensor("sel", [NTILES, 128, R1 + 1, 128], F8,
                         kind="ExternalInput").ap()
    out = nc.dram_tensor("out", [NPC, D], BF, kind="ExternalOutput").ap()

    Relu = mybir.ActivationFunctionType.Relu
    Copy = mybir.ActivationFunctionType.Copy

    with tile.TileContext(nc) as tc:
        with (
            tc.tile_pool(name="wpool", bufs=1) as wpool,
            tc.tile_pool(name="epool", bufs=PF + 3) as epool,
            tc.tile_pool(name="selpool", bufs=PF + 2) as selpool,
            tc.tile_pool(name="ypool", bufs=PSKEW + 2) as ypool,
            tc.tile_pool(name="ypool8", bufs=PSKEW + 2) as ypool8,
            tc.tile_pool(name="opool", bufs=2) as opool,
            tc.tile_pool(name="psy", bufs=PSKEW + 1, space="PSUM") as psy,
            tc.tile_pool(name="pso", bufs=2, space="PSUM") as pso,
        ):
            w8t = [
                wpool.tile([128, b - a, 2, D], F8, name=f"w8t{i}")
                for i, (a, b) in enumerate(zip(WBATCH, WBATCH[1:]))
            ]
            wr8t = [
                wpool.tile([128, b - a, 2, D], F8, name=f"wr8t{i}")
                for i, (a, b) in enumerate(zip(WBATCH, WBATCH[1:]))
            ]

            def load_w(i, which):
                a, b = WBATCH[i], WBATCH[i + 1]
                src = w8i if which == 0 else wr8i
                dst = (w8t if which == 0 else wr8t)[i]
                nc.sync.dma_start(out=dst[:], in_=src[:, a:b])

            def wslice(r, which):
                i = next(j for j in range(len(WBATCH) - 1)
                         if WBATCH[j] <= r < WBATCH[j + 1])
                return (w8t if which == 0 else wr8t)[i][:, r - WBATCH[i]]

            ets, sels = {}, {}

            def alloc_e(t):
                ets[t] = epool.tile([128, 4 * NSLOT], F8, name="imgt")

            def pre_e_seg(t, gi):
                """Load image segment gi of tile t (one contiguous chunk)."""
                segs, _ = _seg_layout(t)
                base, w, has = segs[gi]
                b = base + (4 if has else 2) * w
                nc.sync.dma_start(out=ets[t][:, base:b], in_=img[t, :, base:b])

            def pre_e_full(t):
                # two chunks (split at the GSPLIT[3] segment boundary) so
                # stage-1 can start on the first while the second streams
                segs, total = _seg_layout(t)
                cut = segs[3][0]
                nc.sync.dma_start(out=ets[t][:, :cut], in_=img[t, :, :cut])
                nc.sync.dma_start(out=ets[t][:, cut:total],
                                  in_=img[t, :, cut:total])

            def pre_sel(t):
                sel_t = selpool.tile([128, R1 + 1, 128], F8, name="sel_t")
                nc.sync.dma_start(out=sel_t[:], in_=sel[t])
                sels[t] = sel_t

            def pre_sel_head(t, part):
                """Split sel load for head tiles: small early chunk for the
                first stage-2 pairs, bulk after the stage-1-critical DMAs."""
                if part == 0:
                    sel_t = selpool.tile([128, R1 + 1, 128], F8, name="sel_t")
                    nc.sync.dma_start(out=sel_t[:, :GSPLIT[2]],
                                      in_=sel[t, :, :GSPLIT[2]])
                    sels[t] = sel_t
                else:
                    nc.sync.dma_start(out=sels[t][:, GSPLIT[2]:],
                                      in_=sel[t, :, GSPLIT[2]:])

            def prefetch(t):
                """Whole-tile stream in consumption order (steady state)."""
                if t >= NTILES:
                    return
                alloc_e(t)
                pre_e_full(t)
                pre_sel(t)

            def eblk(t, r, which):
                base, w, off = _img_blk(t, r, which)
                seg = (ets[t][:, base:base + 2 * w]
                       .rearrange("p (c i) -> p c i", c=2))
                return seg[:, :, off:off + P]

            # Startup orchestration: ONE strictly-ordered HWDGE queue (SP)
            # carries W and the head tiles' images in exact consumption
            # order, so DMA_ENGINES delivery can never invert the critical
            # chain.  Tile 0 skips the wr8 term, so wr8 batches stream after
            # the w8/tile-0 head, interleaved with tile 1.
            # Head stream: the LAST segment's data first.  Tile 0's end time
            # is pinned by total head bytes either way; front-loading the
            # tail segment delays the first matmul until the stream can
            # sustain the PE, so the head runs as one long gap-free stretch
            # (each PE idle gap costs ~3us of slow-p-state ramp).
            NSEG = len(GSPLIT) - 1
            alloc_e(0)
            load_w(NSEG - 1, 0)
            load_w(NSEG - 1, 1)
            pre_e_seg(0, NSEG - 1)
            pre_sel(0)
            pre_e_seg(0, NSEG - 2)
            pre_e_seg(0, NSEG - 3)
            for g in range(NSEG - 3):
                load_w(g, 0)
                load_w(g, 1)
                pre_e_seg(0, g)
            load_w(NSEG - 3, 0)
            load_w(NSEG - 3, 1)
            load_w(NSEG - 2, 0)
            load_w(NSEG - 2, 1)
            alloc_e(1)
            for g in range(NSEG):
                pre_e_seg(1, g)
                if g == 0:
                    pre_sel_head(1, 0)
                if g == 3:
                    pre_sel_head(1, 1)

            # continuous pipeline over (tile, pair): stage-1 of the next tile
            # overlaps the stage-2 drain of the previous one.  Tiles 0 and 1
            # interleave at pair granularity so each W batch in the
            # DMA-bound head feeds two tiles' worth of PE work at once.
            order = [(t, m) for t in range(NTILES) for m in range(NPAIR)]
            total = len(order)
            seen = set()
            ys = {}
            outs = {}
            for k in range(total + PSKEW):
                if k < total:
                    t, m = order[k]
                    if t not in seen:
                        seen.add(t)
                        prefetch(t + PF)
                    rels = [r for r in (2 * m, 2 * m + 1) if r < R1]
                    yp = psy.tile([128, 2, D], F32, name="yp")
                    for h, r in enumerate(rels):
                        lE = eblk(t, r, 0)
                        mms = [(lE, wslice(r, 0))]
                        if r < ER8_CUT[t]:
                            mms.append((eblk(t, r, 1), wslice(r, 0)))
                        if r < WR8_CUT[t]:
                            mms.append((lE, wslice(r, 1)))
                        last_h = (h == len(rels) - 1)
                        for j, (lhsT, rhs) in enumerate(mms):
                            nc.tensor.matmul(
                                out=yp[:, h], lhsT=lhsT, rhs=rhs,
                                start=(h == 0 and j == 0),
                                stop=(last_h and j == len(mms) - 1),
                                perf_mode=DR)
                    if m == NPAIR - 1:
                        ets.pop(t)
                    if t in S2F8_TILES:
                        # fp8 Y + always-full evac: the odd final pair's
                        # unused half holds stale-but-finite values that the
                        # zero sel padding column nullifies in stage-2.
                        ysb = ypool8.tile([128, 2, D], F8, name="ysb8")
                        ycopy, ydst = yp[:], ysb[:]
                    else:
                        ysb = ypool.tile([128, 2, D], BF, name="ysb")
                        ycopy = yp[:] if len(rels) == 2 else yp[:, 0]
                        ydst = ysb[:] if len(rels) == 2 else ysb[:, 0]
                    # evac folds the 1/WPRE prescale compensation
                    if k % 2 == 0:
                        nc.vector.tensor_scalar_mul(
                            out=ydst, in0=ycopy, scalar1=1.0 / WPRE)
                    else:
                        nc.scalar.activation(
                            out=ydst, in_=ycopy, func=Copy, scale=1.0 / WPRE)
                    ys[k] = ysb
                if k >= PSKEW:
                    t2, q = order[k - PSKEW]
                    if q == 0:
                        outs[t2] = pso.tile([128, D], F32, name="outp")
                    outp = outs[t2]
                    sel_t2 = sels[t2]
                    ysb_q = ys.pop(k - PSKEW)
                    if t2 in S2F8_TILES:
                        # one DoubleRow matmul reduces both relations of the
                        # pair (K = 256 slots; zero sel pad kills the odd
                        # half of the final pair)
                        nc.tensor.matmul(
                            out=outp[:],
                            lhsT=sel_t2[:, 2 * q:2 * q + 2],
                            rhs=ysb_q[:],
                            start=(q == 0), stop=(q == NPAIR - 1),
                            perf_mode=DR,
                        )
                    else:
                        for h, r in enumerate(
                                [r for r in (2 * q, 2 * q + 1) if r < R1]):
                            nc.tensor.matmul(
                                out=outp[:],
                                lhsT=sel_t2[:, r],
                                rhs=ysb_q[:, h],
                                start=(r == 0), stop=(r == R1 - 1),
                            )
                    if q == NPAIR - 1:
                        outs.pop(t2)
                        sels.pop(t2)
                        osb = opool.tile([128, D], BF)
                        nc.scalar.activation(out=osb[:], in_=outp[:],
                                             func=Relu)
                        base, ncnt = ROW_BASE[t2], CAPS[t2]
                        # last tile: ACT issues its own DMA right after the
                        # relu (no cross-engine semaphore hop on the tail)
                        eng = (nc.scalar if t2 == NTILES - 1 else nc.gpsimd)
                        eng.dma_start(
                            out=out[base:base + ncnt, :], in_=osb[:ncnt, :])

    nc.compile()
    return nc


_NC_CACHE = []


def _get_program():
    if not _NC_CACHE:
        _NC_CACHE.append(build_program())
    return _NC_CACHE[0]


# ---------------------------------------------------------------------------
# Entry point
# ---------------------------------------------------------------------------

def kernel(emb_table, weights, neighbors, relations):
    from concourse import bass_utils

    in_maps, perms = prep(emb_table, weights, neighbors, relations)
    nc = _get_program()
    res = bass_utils.run_bass_kernel_spmd(
        nc, in_maps, core_ids=list(range(NCORES)),
    )
    full = np.zeros((N, D), dtype=np.float32)
    for c in range(NCORES):
        full[perms[c]] = res.results[c]["out"].astype(np.float32)
    return full
